# revision 24
# baseline (speedup 1.0000x reference)
"""Haar DWT2D (pywt even-size convention) on 8 Trainium2 NeuronCores.

Input  x: (16, 64, 512, 512) f32.
Output (LL, (LH, HL, HH)), each (16, 64, 256, 256) f32.

Sharding: pure data parallel over batch — core i handles x[2i:2i+2].

Per-core layout: the (2, 64, 512, 512) slice is viewed flat as N_TILES tiles
of [128 partitions x TILE_FREE f32]. One partition line = TILE_FREE/512
consecutive image rows (even/odd row pairs; the row count divides 512 so a
line never straddles images and always starts on an even row). Two-stage
butterfly per tile, all on DVE:
  stage 1:  Oh = 0.5*O; S = 0.5*E + Oh; D = 0.5*E - Oh   (row direction)
  stage 2:  LL = Se + So, HL = Se - So, LH = De + Do, HH = De - Do
            (column direction, stride-2 reads / contiguous writes)
The four outputs are packed side by side in one [128, TILE_FREE] SBUF tile
and leave in a single DMA per tile to a packed y[4, ...] DRAM tensor.

Raw bass (no Tile): walrus' CoreV2/V3 descriptors have a single embedded
sync-wait slot, so all waits are standalone wait_ge instructions on the
issuing engine. SP sequencer does input DMAs, ACT does output DMAs, DVE
computes; NB-deep buffers decouple the three.
"""

from contextlib import ExitStack

import numpy as np

from concourse import bass, mybir
from concourse.bass_utils import run_bass_kernel_spmd

N_CORES = 8
B, C, H, W = 16, 64, 512, 512
PER_CORE_B = B // N_CORES  # 2
TILE_FREE = 8192  # f32 per partition line (32 KB) -> 4 MiB per DMA
N_TILES = PER_CORE_B * C * H * W // (128 * TILE_FREE)  # 64
OUT_NAMES = ("ll", "lh", "hl", "hh")

FP32 = mybir.dt.float32
NB = 2  # buffer depth for X and OUT tiles
NSD = 1  # S/D/Oh buffers (same-engine producer/consumer: 1 is enough)


def _build_kernel(n_tiles: int = N_TILES) -> bass.Bass:
    nc = bass.Bass()
    x = nc.dram_tensor("x", [n_tiles, 128, TILE_FREE], FP32, kind="ExternalInput")
    # [tile, partition, quadrant, column]: each partition's 4 quadrant rows
    # are contiguous in DRAM, so the out-DMA is one contiguous line per
    # partition; quadrants are separated on the host afterwards.
    y = nc.dram_tensor(
        "y", [n_tiles, 128, 4, TILE_FREE // 4], FP32, kind="ExternalOutput"
    )
    y_view = y[:]

    with ExitStack() as ctx:
        X = [
            ctx.enter_context(nc.sbuf_tensor(f"xb{i}", [128, TILE_FREE], FP32))
            for i in range(NB)
        ]
        OUT = [
            ctx.enter_context(nc.sbuf_tensor(f"ob{i}", [128, TILE_FREE], FP32))
            for i in range(NB)
        ]
        S = [
            ctx.enter_context(
                nc.sbuf_tensor(f"sb{i}", [128, TILE_FREE // 2], FP32)
            )
            for i in range(NSD)
        ]
        D = [
            ctx.enter_context(
                nc.sbuf_tensor(f"db{i}", [128, TILE_FREE // 2], FP32)
            )
            for i in range(NSD)
        ]
        OH = [
            ctx.enter_context(
                nc.sbuf_tensor(f"oh{i}", [128, TILE_FREE // 2], FP32)
            )
            for i in range(NSD)
        ]
        # one in/out semaphore per buffer slot: a slot's DMAs are serialized
        # by the slot-reuse dependency, so each sem only ever counts one
        # in-flight DMA (two concurrent DMAs on one counting sem would race:
        # 16 increments could be 8+8 from two half-done transfers)
        sem_in = [
            ctx.enter_context(nc.semaphore(f"sem_in{i}")) for i in range(NB)
        ]
        sem_out = [
            ctx.enter_context(nc.semaphore(f"sem_out{i}")) for i in range(NB)
        ]
        # sem_v: one increment per tile when D' retires — the DVE pipeline
        # has no RAW interlock between instructions, so stage 2 waits on it;
        # it also marks X and Oh fully consumed (SP / ACT slot reuse).
        # sem_act: Oh written by ACT, read by DVE's S'/D'.
        # sem_dve: stage 2 (OUT tile) retired, gates the out-DMA.
        sem_v = ctx.enter_context(nc.semaphore("sem_v"))
        sem_act = ctx.enter_context(nc.semaphore("sem_act"))
        sem_dve = ctx.enter_context(nc.semaphore("sem_dve"))
        block = ctx.enter_context(nc.Block())

        def out_dma(scalar, t):
            i = t % NB
            scalar.wait_ge(sem_dve, t + 1)
            if t >= NB:
                scalar.wait_ge(sem_out[i], 16 * (t // NB))
            ob = OUT[i][:].rearrange("p (q m) -> p q m", m=TILE_FREE // 4)
            scalar.dma_start(out=y_view[t], in_=ob).then_inc(sem_out[i], 16)

        @block.sync
        def _(sync):
            for t in range(n_tiles):
                i = t % NB
                if t >= NB:
                    # X[i] was last read by DVE stage 1 and ACT's Oh of
                    # iteration t - NB
                    sync.wait_ge(sem_v, t - NB + 1)
                    sync.wait_ge(sem_act, t - NB + 1)
                    # observe this slot's previous DMA completion (implied
                    # by the waits above, but the sem update rule wants the
                    # issuing engine to have seen the current value)
                    sync.wait_ge(sem_in[i], 16 * (t // NB))
                sync.dma_start(out=X[i][:], in_=x[t]).then_inc(sem_in[i], 16)

        @block.scalar
        def _(scalar):
            # ACT: compute Oh(t) = 0.5 * O(t) early each iteration, then
            # issue the out-DMA for tile t-1. Oh(t) only needs X(t) and the
            # retire of D'(t-1), so it never waits behind DVE's stage 2.
            for t in range(n_tiles):
                Oht = OH[t % NSD]
                scalar.wait_ge(sem_in[t % NB], 16 * (t // NB + 1))
                if t >= 1:
                    scalar.wait_ge(sem_v, t)  # D'(t-1) retired: Oh free
                xv = X[t % NB][:].rearrange("p (k t m) -> p k t m", t=2, m=512)
                Ov = xv[:, :, 1, :]
                Ohv = Oht[:].rearrange("p (k m) -> p k m", m=512)
                nc.scalar.mul(Ohv, Ov, 0.5).then_inc(sem_act, 1)
                if t >= 1:
                    out_dma(scalar, t - 1)
            out_dma(scalar, n_tiles - 1)
            # drain: don't let the kernel end with output DMAs in flight
            for i in range(NB):
                n_dmas = len(range(i, n_tiles, NB))
                if n_dmas:
                    scalar.wait_ge(sem_out[i], 16 * n_dmas)

        @block.vector
        def _(vector):
            for t in range(n_tiles):
                Xt = X[t % NB]
                St, Dt, Oht = S[t % NSD], D[t % NSD], OH[t % NSD]
                Ot = OUT[t % NB]

                vector.wait_ge(sem_in[t % NB], 16 * (t // NB + 1))
                vector.wait_ge(sem_act, t + 1)  # Oh(t) ready
                if t >= 1:
                    # stage 2 of t-1 retired: its reads of S/D are done, so
                    # S'/D'(t) may overwrite them (same-engine WAR needs an
                    # explicit retire wait too)
                    vector.wait_ge(sem_dve, t)
                xv = Xt[:].rearrange("p (k t m) -> p k t m", t=2, m=512)
                E = xv[:, :, 0, :]
                Ohv = Oht[:].rearrange("p (k m) -> p k m", m=512)
                Sv = St[:].rearrange("p (k m) -> p k m", m=512)
                Dv = Dt[:].rearrange("p (k m) -> p k m", m=512)
                # S = 0.5*E + Oh, D = 0.5*E - Oh (Oh = 0.5*O from ACT)
                nc.vector.scalar_tensor_tensor(
                    Sv, E, 0.5, Ohv, mybir.AluOpType.mult, mybir.AluOpType.add
                )
                nc.vector.scalar_tensor_tensor(
                    Dv, E, 0.5, Ohv, mybir.AluOpType.mult, mybir.AluOpType.subtract
                ).then_inc(sem_v, 1)
                vector.wait_ge(sem_v, t + 1)  # S'/D' retired (in order)

                if t >= NB:
                    # OUT[t % NB] was drained by the out-DMA of t - NB
                    vector.wait_ge(sem_out[t % NB], 16 * (t // NB))
                Sp = St[:].rearrange("p (k m t) -> p k t m", t=2, m=256)
                Dp = Dt[:].rearrange("p (k m t) -> p k t m", t=2, m=256)
                Se, So = Sp[:, :, 0, :], Sp[:, :, 1, :]
                De, Do = Dp[:, :, 0, :], Dp[:, :, 1, :]
                # quadrant order matches OUT_NAMES / y's q axis
                ov = Ot[:].rearrange("p (q k m) -> p q k m", q=4, m=256)
                nc.vector.tensor_add(ov[:, 0], Se, So)  # LL
                nc.vector.tensor_add(ov[:, 1], De, Do)  # LH
                nc.vector.tensor_sub(ov[:, 2], Se, So)  # HL
                nc.vector.tensor_sub(ov[:, 3], De, Do).then_inc(sem_dve, 1)  # HH

    return nc


def _shard_inputs(x: np.ndarray) -> list[dict[str, np.ndarray]]:
    x = np.ascontiguousarray(np.asarray(x, dtype=np.float32))
    assert x.shape == (B, C, H, W), x.shape
    return [
        {
            "x": x[i * PER_CORE_B : (i + 1) * PER_CORE_B].reshape(
                N_TILES, 128, TILE_FREE
            )
        }
        for i in range(N_CORES)
    ]


def _gather(results: list[dict[str, np.ndarray]]) -> dict[str, np.ndarray]:
    full = {}
    for q, name in enumerate(OUT_NAMES):
        full[name] = np.concatenate(
            [
                np.ascontiguousarray(results[i]["y"][:, :, q, :]).reshape(
                    PER_CORE_B, C, H // 2, W // 2
                )
                for i in range(N_CORES)
            ],
            axis=0,
        )
    return full


def _run(x: np.ndarray, **spmd_kwargs):
    nc = _build_kernel()
    in_maps = _shard_inputs(x)
    out = run_bass_kernel_spmd(nc, in_maps, list(range(N_CORES)), **spmd_kwargs)
    return _gather(out.results), out


def kernel(x: np.ndarray):
    full, _ = _run(x)
    return (full["ll"], (full["lh"], full["hl"], full["hh"]))


# revision 26
# speedup vs baseline: 1.2652x; 1.2652x over previous
"""Haar DWT2D (pywt even-size convention) on 8 Trainium2 NeuronCores.

Input  x: (16, 64, 512, 512) f32.
Output (LL, (LH, HL, HH)), each (16, 64, 256, 256) f32.

Sharding: pure data parallel over batch — core i handles x[2i:2i+2].

Per-core layout: the (2, 64, 512, 512) slice is viewed flat as N_TILES tiles
of [128 partitions x TILE_FREE f32]. One partition line = TILE_FREE/512
consecutive image rows (even/odd row pairs; the row count divides 512 so a
line never straddles images and always starts on an even row). Two-stage
butterfly per tile, all on DVE:
  stage 1:  Oh = 0.5*O; S = 0.5*E + Oh; D = 0.5*E - Oh   (row direction)
  stage 2:  LL = Se + So, HL = Se - So, LH = De + Do, HH = De - Do
            (column direction, stride-2 reads / contiguous writes)
The four outputs are packed side by side in one [128, TILE_FREE] SBUF tile
and leave in a single DMA per tile to a packed y[4, ...] DRAM tensor.

Raw bass (no Tile): walrus' CoreV2/V3 descriptors have a single embedded
sync-wait slot, so all waits are standalone wait_ge instructions on the
issuing engine. SP sequencer does input DMAs, ACT does output DMAs, DVE
computes; NB-deep buffers decouple the three.
"""

from contextlib import ExitStack

import numpy as np

from concourse import bass, mybir
from concourse.bass_utils import run_bass_kernel_spmd

N_CORES = 8
B, C, H, W = 16, 64, 512, 512
PER_CORE_B = B // N_CORES  # 2
TILE_FREE = 8192  # f32 per partition line (32 KB) -> 4 MiB per DMA
N_TILES = PER_CORE_B * C * H * W // (128 * TILE_FREE)  # 64
OUT_NAMES = ("ll", "lh", "hl", "hh")

FP32 = mybir.dt.float32
NB = 2  # buffer depth for X and OUT tiles
NSD = 1  # S/D/Oh buffers (same-engine producer/consumer: 1 is enough)


def _build_kernel(n_tiles: int = N_TILES) -> bass.Bass:
    nc = bass.Bass()
    x = nc.dram_tensor("x", [n_tiles, 128, TILE_FREE], FP32, kind="ExternalInput")
    # Quadrant-major output: the out-DMA writes 4 chunks of TILE_FREE/4 f32
    # per partition to 4 separate DRAM regions. The smaller (8 KB)
    # descriptors interleave measurably better with the 32 KB input
    # descriptors than one contiguous 32 KB line per partition would
    # (A/B measured: 684 us vs 835 us).
    y = nc.dram_tensor(
        "y", [4, n_tiles, 128, TILE_FREE // 4], FP32, kind="ExternalOutput"
    )
    # DMA view: for tile t, [partition, quadrant, column]
    y_view = y[:].rearrange("q n p m -> n p q m")

    with ExitStack() as ctx:
        X = [
            ctx.enter_context(nc.sbuf_tensor(f"xb{i}", [128, TILE_FREE], FP32))
            for i in range(NB)
        ]
        OUT = [
            ctx.enter_context(nc.sbuf_tensor(f"ob{i}", [128, TILE_FREE], FP32))
            for i in range(NB)
        ]
        S = [
            ctx.enter_context(
                nc.sbuf_tensor(f"sb{i}", [128, TILE_FREE // 2], FP32)
            )
            for i in range(NSD)
        ]
        D = [
            ctx.enter_context(
                nc.sbuf_tensor(f"db{i}", [128, TILE_FREE // 2], FP32)
            )
            for i in range(NSD)
        ]
        OH = [
            ctx.enter_context(
                nc.sbuf_tensor(f"oh{i}", [128, TILE_FREE // 2], FP32)
            )
            for i in range(NSD)
        ]
        # one in/out semaphore per buffer slot: a slot's DMAs are serialized
        # by the slot-reuse dependency, so each sem only ever counts one
        # in-flight DMA (two concurrent DMAs on one counting sem would race:
        # 16 increments could be 8+8 from two half-done transfers)
        sem_in = [
            ctx.enter_context(nc.semaphore(f"sem_in{i}")) for i in range(NB)
        ]
        sem_out = [
            ctx.enter_context(nc.semaphore(f"sem_out{i}")) for i in range(NB)
        ]
        # sem_v: one increment per tile when D' retires — the DVE pipeline
        # has no RAW interlock between instructions, so stage 2 waits on it;
        # it also marks X and Oh fully consumed (SP / ACT slot reuse).
        # sem_act: Oh written by ACT, read by DVE's S'/D'.
        # sem_dve: stage 2 (OUT tile) retired, gates the out-DMA.
        sem_v = ctx.enter_context(nc.semaphore("sem_v"))
        sem_act = ctx.enter_context(nc.semaphore("sem_act"))
        sem_dve = ctx.enter_context(nc.semaphore("sem_dve"))
        block = ctx.enter_context(nc.Block())

        def out_dma(scalar, t):
            i = t % NB
            scalar.wait_ge(sem_dve, t + 1)
            if t >= NB:
                scalar.wait_ge(sem_out[i], 16 * (t // NB))
            ob = OUT[i][:].rearrange("p (q m) -> p q m", m=TILE_FREE // 4)
            scalar.dma_start(out=y_view[t], in_=ob).then_inc(sem_out[i], 16)

        @block.sync
        def _(sync):
            for t in range(n_tiles):
                i = t % NB
                if t >= NB:
                    # X[i] was last read by DVE stage 1 and ACT's Oh of
                    # iteration t - NB
                    sync.wait_ge(sem_v, t - NB + 1)
                    sync.wait_ge(sem_act, t - NB + 1)
                    # observe this slot's previous DMA completion (implied
                    # by the waits above, but the sem update rule wants the
                    # issuing engine to have seen the current value)
                    sync.wait_ge(sem_in[i], 16 * (t // NB))
                sync.dma_start(out=X[i][:], in_=x[t]).then_inc(sem_in[i], 16)

        @block.scalar
        def _(scalar):
            # ACT: compute Oh(t) = 0.5 * O(t) early each iteration, then
            # issue the out-DMA for tile t-1. Oh(t) only needs X(t) and the
            # retire of D'(t-1), so it never waits behind DVE's stage 2.
            for t in range(n_tiles):
                Oht = OH[t % NSD]
                scalar.wait_ge(sem_in[t % NB], 16 * (t // NB + 1))
                if t >= 1:
                    scalar.wait_ge(sem_v, t)  # D'(t-1) retired: Oh free
                xv = X[t % NB][:].rearrange("p (k t m) -> p k t m", t=2, m=512)
                Ov = xv[:, :, 1, :]
                Ohv = Oht[:].rearrange("p (k m) -> p k m", m=512)
                nc.scalar.mul(Ohv, Ov, 0.5).then_inc(sem_act, 1)
                if t >= 1:
                    out_dma(scalar, t - 1)
            out_dma(scalar, n_tiles - 1)
            # drain: don't let the kernel end with output DMAs in flight
            for i in range(NB):
                n_dmas = len(range(i, n_tiles, NB))
                if n_dmas:
                    scalar.wait_ge(sem_out[i], 16 * n_dmas)

        @block.vector
        def _(vector):
            for t in range(n_tiles):
                Xt = X[t % NB]
                St, Dt, Oht = S[t % NSD], D[t % NSD], OH[t % NSD]
                Ot = OUT[t % NB]

                vector.wait_ge(sem_in[t % NB], 16 * (t // NB + 1))
                vector.wait_ge(sem_act, t + 1)  # Oh(t) ready
                if t >= 1:
                    # stage 2 of t-1 retired: its reads of S/D are done, so
                    # S'/D'(t) may overwrite them (same-engine WAR needs an
                    # explicit retire wait too)
                    vector.wait_ge(sem_dve, t)
                xv = Xt[:].rearrange("p (k t m) -> p k t m", t=2, m=512)
                E = xv[:, :, 0, :]
                Ohv = Oht[:].rearrange("p (k m) -> p k m", m=512)
                Sv = St[:].rearrange("p (k m) -> p k m", m=512)
                Dv = Dt[:].rearrange("p (k m) -> p k m", m=512)
                # S = 0.5*E + Oh, D = 0.5*E - Oh (Oh = 0.5*O from ACT)
                nc.vector.scalar_tensor_tensor(
                    Sv, E, 0.5, Ohv, mybir.AluOpType.mult, mybir.AluOpType.add
                )
                nc.vector.scalar_tensor_tensor(
                    Dv, E, 0.5, Ohv, mybir.AluOpType.mult, mybir.AluOpType.subtract
                ).then_inc(sem_v, 1)
                vector.wait_ge(sem_v, t + 1)  # S'/D' retired (in order)

                if t >= NB:
                    # OUT[t % NB] was drained by the out-DMA of t - NB
                    vector.wait_ge(sem_out[t % NB], 16 * (t // NB))
                Sp = St[:].rearrange("p (k m t) -> p k t m", t=2, m=256)
                Dp = Dt[:].rearrange("p (k m t) -> p k t m", t=2, m=256)
                Se, So = Sp[:, :, 0, :], Sp[:, :, 1, :]
                De, Do = Dp[:, :, 0, :], Dp[:, :, 1, :]
                # quadrant order matches OUT_NAMES / y's q axis
                ov = Ot[:].rearrange("p (q k m) -> p q k m", q=4, m=256)
                nc.vector.tensor_add(ov[:, 0], Se, So)  # LL
                nc.vector.tensor_add(ov[:, 1], De, Do)  # LH
                nc.vector.tensor_sub(ov[:, 2], Se, So)  # HL
                nc.vector.tensor_sub(ov[:, 3], De, Do).then_inc(sem_dve, 1)  # HH

    return nc


def _shard_inputs(x: np.ndarray) -> list[dict[str, np.ndarray]]:
    x = np.ascontiguousarray(np.asarray(x, dtype=np.float32))
    assert x.shape == (B, C, H, W), x.shape
    return [
        {
            "x": x[i * PER_CORE_B : (i + 1) * PER_CORE_B].reshape(
                N_TILES, 128, TILE_FREE
            )
        }
        for i in range(N_CORES)
    ]


def _gather(results: list[dict[str, np.ndarray]]) -> dict[str, np.ndarray]:
    full = {}
    for q, name in enumerate(OUT_NAMES):
        full[name] = np.concatenate(
            [
                results[i]["y"][q].reshape(PER_CORE_B, C, H // 2, W // 2)
                for i in range(N_CORES)
            ],
            axis=0,
        )
    return full


def _run(x: np.ndarray, **spmd_kwargs):
    nc = _build_kernel()
    in_maps = _shard_inputs(x)
    out = run_bass_kernel_spmd(nc, in_maps, list(range(N_CORES)), **spmd_kwargs)
    return _gather(out.results), out


def kernel(x: np.ndarray):
    full, _ = _run(x)
    return (full["ll"], (full["lh"], full["hl"], full["hh"]))
